# revision 16
# baseline (speedup 1.0000x reference)
"""GATv2 (2-layer, 8-head) Trainium2 kernel, 8-core node-sharded.

Pipeline per layer:
  T-NEFF (per-core, sharded): node transforms xl = x@Wl+bl, xr = x@Wr+br
    via bf16 matmuls with chunked (4-block) DMA; emits bf16 gather tables
    (xl) and bf16 xr shards.  Feature columns are stored (c, h)-interleaved
    so downstream broadcasts keep innermost stride 1.
  host: assembles the full xl gather table from the 8 shards (data movement
    only), then
  E-NEFF (per-core, sharded by dst): per-edge score + segment-softmax +
    aggregate, with edges laid out stratum-major: edge slot (q, d) holds the
    q-th in-edge of dst-slot d, so partition index == dst slot.  Per block:
      Pool   gpsimd dma_gather of xl[src] rows (bf16, 256B rows)
      DVE    tt = slab + xr (broadcast add, 2x mode)
      SP-DMA xbar transpose tt -> ttT (features on partitions)
      Act    uT = |ttT|
      PE     per-stratum score matmuls: S = ttT'@(0.6 att) + |ttT|'@(0.4 att)
             (leaky_relu(z) = 0.6 z + 0.4 |z|), dst-partition [128, 8] outs
      Act    exv = exp(S)  (no max-subtraction; scores are O(10))
      DVE    exv *= pad mask; Ms = slab * exv  (per-head alpha-weighting)
      PE     segment sum via PSUM-accumulated identity matmuls (agg | denom)
      DVE    out = agg/denom + bias (+ELU for layer 1)
  Layer 2 ends with the log-softmax tail (ln via exponent/mantissa poly).

Between the two layers the host only concatenates/permutes shards.
"""

import os
from contextlib import ExitStack

import ml_dtypes
import numpy as np

N, E0, DIN, H, DH, DOUT = 50000, 1600000, 128, 8, 16, 7
F1 = H * DH            # 128
C2 = 8                 # layer-2 per-head padded cols (7 real)
F2P = H * C2           # 64
NCORES = 8
P = 128
NBLK = 392             # 392*128 = 50176 >= N, 392 % 8 == 0
NB = NBLK // NCORES    # 49 blocks per core
NOWN = NB * P          # 6272 nodes per core (incl. pad slots)
NPAD = NBLK * P        # 50176
SPLIT = 32768
TABB_ROWS = NPAD - SPLIT  # 17408
EPS = 1e-16
SB = 7                 # blocks per prefetch super-block (49 = 7*7)

_f32 = np.float32
_bf16 = ml_dtypes.bfloat16


# ---------------------------------------------------------------------------
# host-side graph preprocessing (pure index/layout manipulation)
# ---------------------------------------------------------------------------

def _prep_graph(edge_index):
    src = np.concatenate([edge_index[0], np.arange(N, dtype=np.int64)])
    dst = np.concatenate([edge_index[1], np.arange(N, dtype=np.int64)])
    src = src.astype(np.int64)
    dst = dst.astype(np.int64)

    low = src < SPLIT
    l_cnt = np.bincount(dst[low], minlength=N).astype(np.int64)
    h_cnt = np.bincount(dst[~low], minlength=N).astype(np.int64)

    # group nodes into blocks of 128 with near-equal (low-deg, high-deg)
    order = np.lexsort((h_cnt, l_cnt))
    nodes_sorted = np.concatenate([order, np.full(NPAD - N, -1, np.int64)])
    blocks = nodes_sorted.reshape(NBLK, P)          # [392, 128]

    l_blk = np.where(blocks >= 0, l_cnt[np.maximum(blocks, 0)], 0).max(axis=1)
    h_blk = np.where(blocks >= 0, h_cnt[np.maximum(blocks, 0)], 0).max(axis=1)
    # block-slot j on every core runs global blocks j*8+k; shared strata counts
    GA = l_blk.reshape(NB, NCORES).max(axis=1).astype(int)   # [49]
    GB = h_blk.reshape(NB, NCORES).max(axis=1).astype(int)
    # round up to even so g=GA+GB is even (layer-2 stratum pairing) and
    # dma_gather's num_idxs register values stay few
    GA = np.where(GA > 0, (GA + 1) & ~1, 0)
    GB = np.where(GB > 0, (GB + 1) & ~1, 0)

    # per-node padded src lists, split by src half
    key = dst * 2 + (~low).astype(np.int64)
    oe = np.argsort(key, kind="stable")
    ss, sk = src[oe], key[oe]
    starts = np.searchsorted(sk, np.arange(2 * N))
    pos = np.arange(len(ss)) - starts[sk]
    Amax = max(int(l_cnt.max()), int(GA.max()))
    Bmax = max(int(h_cnt.max()), int(GB.max()))
    A_pad = np.zeros((N, Amax), np.int32)
    B_pad = np.zeros((N, Bmax), np.int32)
    am = (sk % 2) == 0
    A_pad[sk[am] // 2, pos[am]] = ss[am]
    B_pad[sk[~am] // 2, pos[~am]] = ss[~am] - SPLIT

    sumGA, sumGB = int(GA.sum()), int(GB.sum())
    sumG = sumGA + sumGB

    members = [None] * NCORES
    idxA = [None] * NCORES
    idxB = [None] * NCORES
    mk01 = [None] * NCORES

    for k in range(NCORES):
        mem = blocks[np.arange(NB) * NCORES + k]       # [49, 128]
        members[k] = mem
        ia = np.zeros((P, 8 * sumGA), np.int16)
        ib = np.zeros((P, 8 * sumGB), np.int16)
        mg = np.zeros((P, sumG, H), _bf16)
        oa = ob = om = 0
        for j in range(NB):
            ga, gb = GA[j], GB[j]
            m = mem[j]
            msafe = np.maximum(m, 0)
            larr = np.where(m >= 0, l_cnt[msafe], 0)
            harr = np.where(m >= 0, h_cnt[msafe], 0)
            if ga:
                plane = A_pad[msafe, :ga]              # [128, ga] (d, q)
                flat = plane.T.reshape(-1)             # slot-major (q, d)
                ia[:, 8 * oa:8 * (oa + ga)] = np.tile(
                    flat.reshape(-1, 16).T, (8, 1)).astype(np.int16)
                mg[:, om:om + ga, :] = (
                    np.arange(ga)[None, :] < larr[:, None]
                ).astype(_bf16)[:, :, None]
            if gb:
                plane = B_pad[msafe, :gb]
                flat = plane.T.reshape(-1)
                ib[:, 8 * ob:8 * (ob + gb)] = np.tile(
                    flat.reshape(-1, 16).T, (8, 1)).astype(np.int16)
                mg[:, om + ga:om + ga + gb, :] = (
                    np.arange(gb)[None, :] < harr[:, None]
                ).astype(_bf16)[:, :, None]
            oa += ga
            ob += gb
            om += ga + gb
        idxA[k], idxB[k], mk01[k] = ia, ib, mg.reshape(P, sumG * H)

    return dict(members=members, GA=GA, GB=GB, idxA=idxA, idxB=idxB,
                mk01=mk01, sumGA=sumGA, sumGB=sumGB, sumG=sumG)


# ---------------------------------------------------------------------------
# NEFF builders
# ---------------------------------------------------------------------------

def _mk_bass():
    import concourse.bacc as bacc
    return bacc.Bacc("TRN2", target_bir_lowering=False)


def _build_transform(fo, xl_cols, xl_w, xr_w):
    """xT [128, NOWN] @ Wcat [128, fo] -> xl rows (bf16) + xr rows (bf16).

    xl tensor is [NOWN, xl_cols]; only cols [0:xl_w] are written (rest
    stays zero).  xr tensor is [NOWN, xr_w].  IO in 4-block chunks."""
    import concourse.mybir as mybir
    import concourse.tile as tile

    CH = 4                       # blocks per chunk
    NCH = NB // CH               # 12 full chunks
    REM = NB - NCH * CH          # 1 leftover block

    nc = _mk_bass()
    BF16, F32 = mybir.dt.bfloat16, mybir.dt.float32
    op = mybir.AluOpType
    xT = nc.dram_tensor("xT", [P, NOWN], F32, kind="ExternalInput")
    W = nc.dram_tensor("Wcat", [P, fo], F32, kind="ExternalInput")
    B = nc.dram_tensor("Bcat", [P, fo], F32, kind="ExternalInput")
    xl = nc.dram_tensor("xl", [NOWN, xl_cols], BF16, kind="ExternalOutput")
    xr = nc.dram_tensor("xr", [NOWN, xr_w], BF16, kind="ExternalOutput")

    with tile.TileContext(nc) as tc, ExitStack() as ctx:
        const = ctx.enter_context(tc.tile_pool(name="const", bufs=1))
        work = ctx.enter_context(tc.tile_pool(name="work", bufs=3))
        psum = ctx.enter_context(tc.tile_pool(name="psum", bufs=4, space="PSUM"))

        w_f = const.tile([P, fo], F32)
        nc.sync.dma_start(w_f[:], W[:, :])
        w_s = const.tile([P, fo], BF16)
        nc.vector.tensor_copy(w_s[:], w_f[:])
        b_s = const.tile([P, fo], F32)
        nc.sync.dma_start(b_s[:], B[:, :])

        def chunk(c0, nb):
            rows = nb * P
            lhs_f = work.tile([P, nb, P], F32, tag="lhsf")
            nc.sync.dma_start(
                lhs_f[:], xT[:, c0 * P:c0 * P + rows].rearrange(
                    "p (b q) -> p b q", q=P))
            lhs = work.tile([P, nb, P], BF16, tag="lhs")
            nc.vector.tensor_copy(lhs[:], lhs_f[:])
            ol = work.tile([P, nb, xl_w], BF16, tag="ol")
            orr = work.tile([P, nb, xr_w], BF16, tag="orr")
            for b in range(nb):
                ps = psum.tile([P, fo], F32, tag="ps")
                nc.tensor.matmul(ps[:], lhs[:, b, :], w_s[:],
                                 start=True, stop=True)
                nc.vector.tensor_tensor(ol[:, b, :], ps[:, 0:xl_w],
                                        b_s[:, 0:xl_w], op.add)
                nc.vector.tensor_tensor(orr[:, b, :], ps[:, xl_w:fo],
                                        b_s[:, xl_w:fo], op.add)
            nc.sync.dma_start(
                xl[c0 * P:c0 * P + rows, 0:xl_w].rearrange(
                    "(b p) f -> p b f", p=P), ol[:])
            nc.sync.dma_start(
                xr[c0 * P:c0 * P + rows, :].rearrange(
                    "(b p) f -> p b f", p=P), orr[:])

        for c in range(NCH):
            chunk(c * CH, CH)
        if REM:
            chunk(NCH * CH, REM)
    nc.compile()
    return nc


def _build_edge(layer, GA, GB, sumGA, sumGB, sumG):
    """Edge phase for one layer (see module docstring)."""
    import concourse.bass as bass
    import concourse.mybir as mybir
    import concourse.tile as tile
    from concourse import library_config

    FU = F1 if layer == 1 else F2P      # used feature cols (128 / 64)
    C = DH if layer == 1 else C2        # per-head cols (16 / 8)
    FOUT = F1 if layer == 1 else F2P
    PAIR = 1 if layer == 1 else 2       # strata per transposed 128-col group
    AW = 8 * PAIR                       # att rhs width per pass
    # layer 1: tt = slab + xr computed in place (slab IS tt afterwards;
    # aggregation of ex*(xl+xr) is corrected by -xr*denom/(denom+eps)).
    # layer 2: slab cols 0:64 -> compact tt tile (xbar needs contiguous in).
    INPLACE = layer == 1

    nc = _mk_bass()
    dt = mybir.dt
    op = mybir.AluOpType
    AF = mybir.ActivationFunctionType

    tabA = nc.dram_tensor("tabA", [SPLIT, P], dt.bfloat16, kind="ExternalInput")
    tabB = nc.dram_tensor("tabB", [TABB_ROWS, P], dt.bfloat16, kind="ExternalInput")
    xr_d = nc.dram_tensor("xr", [NOWN, FU], dt.bfloat16, kind="ExternalInput")
    idxA = nc.dram_tensor("idxA", [P, 8 * sumGA], dt.int16, kind="ExternalInput")
    idxB = nc.dram_tensor("idxB", [P, 8 * sumGB], dt.int16, kind="ExternalInput")
    mk01 = nc.dram_tensor("mk01", [P, sumG * H], dt.bfloat16, kind="ExternalInput")
    attc = nc.dram_tensor("attc", [P, 2 * AW], dt.bfloat16, kind="ExternalInput")
    biasT = nc.dram_tensor("biasT", [P, FU], dt.float32, kind="ExternalInput")
    idT = nc.dram_tensor("idT", [P, P], dt.bfloat16, kind="ExternalInput")
    out_d = nc.dram_tensor("out", [NOWN, FOUT], dt.float32, kind="ExternalOutput")

    with tile.TileContext(nc) as tc, ExitStack() as ctx:
        const = ctx.enter_context(tc.tile_pool(name="const", bufs=1))
        pref = ctx.enter_context(tc.tile_pool(name="pref", bufs=2))
        slabp = ctx.enter_context(tc.tile_pool(name="slabp", bufs=5))
        tpose = ctx.enter_context(tc.tile_pool(name="tpose", bufs=2))
        exvp = ctx.enter_context(tc.tile_pool(name="exvp", bufs=4))
        psum = ctx.enter_context(tc.tile_pool(name="psum", bufs=2, space="PSUM"))
        spsum = ctx.enter_context(tc.tile_pool(name="spsum", bufs=4, space="PSUM"))
        small = ctx.enter_context(tc.tile_pool(name="small", bufs=3))

        nc.gpsimd.load_library(library_config.mlp)

        regcache = {}

        def nreg(v):
            if v not in regcache:
                regcache[v] = nc.gpsimd.to_reg(v)
            return regcache[v]

        att_s = const.tile([P, 2 * AW], dt.bfloat16)
        nc.sync.dma_start(att_s[:], attc[:, :])
        bias_s = const.tile([P, FU], dt.float32)
        nc.sync.dma_start(bias_s[:], biasT[:, :])
        id_s = const.tile([P, P], dt.bfloat16)
        nc.sync.dma_start(id_s[:], idT[:, :])

        if layer == 2:
            persist = ctx.enter_context(tc.tile_pool(name="persist", bufs=1))
            mx_all = persist.tile([P, NB], dt.float32)
            s_all = persist.tile([P, NB], dt.float32)
            y_tiles = [None] * NB

        # block table: per block j -> (ga, gb, idx offsets, superblock id)
        binfo = []
        oa = obi = om = 0
        sb_starts = {}
        for j0 in range(0, NB, SB):
            jb = min(SB, NB - j0)
            sa = int(GA[j0:j0 + jb].sum())
            sb_ = int(GB[j0:j0 + jb].sum())
            sg = sa + sb_
            sb_starts[j0] = (oa, obi, om, sa, sb_, sg)
            la = lb = lg = 0
            for bi in range(jb):
                jj = j0 + bi
                binfo.append((jj, j0, bi, la, lb, lg,
                              int(GA[jj]), int(GB[jj])))
                la += int(GA[jj])
                lb += int(GB[jj])
                lg += int(GA[jj]) + int(GB[jj])
            oa += sa
            obi += sb_
            om += sg

        prefs = {}           # superblock j0 -> (ia_t, ib_t, mk_t, xr_t)
        state = {}           # block jj -> dict of tiles for later stages

        def do_pref(j0):
            oa0, ob0, om0, sa, sb_, sg = sb_starts[j0]
            jb = min(SB, NB - j0)
            ia_t = ib_t = None
            if sa:
                ia_t = pref.tile([P, 8 * sa], dt.int16, tag="ia", name="ia_t")
                nc.sync.dma_start(ia_t[:], idxA[:, 8 * oa0:8 * (oa0 + sa)])
            if sb_:
                ib_t = pref.tile([P, 8 * sb_], dt.int16, tag="ib", name="ib_t")
                nc.sync.dma_start(ib_t[:], idxB[:, 8 * ob0:8 * (ob0 + sb_)])
            mk_t = pref.tile([P, sg, H], dt.bfloat16, tag="mk", name="mk_t")
            nc.sync.dma_start(
                mk_t[:], mk01[:, om0 * H:(om0 + sg) * H].rearrange(
                    "p (g h) -> p g h", h=H))
            xr_t = pref.tile([P, jb, FU], dt.bfloat16, tag="xr", name="xr_t")
            nc.sync.dma_start(
                xr_t[:], xr_d[j0 * P:(j0 + jb) * P, :].rearrange(
                    "(b p) f -> p b f", p=P))
            prefs[j0] = (ia_t, ib_t, mk_t, xr_t)

        CHKS = 16                # strata per pipeline chunk

        def p1(blk):
            """gather; per chunk: add, xbar, abs, score mms, exp."""
            jj, j0, bi, la, lb, lg, ga, gb = blk
            if jj == 0:
                do_pref(0)
                if SB < NB:
                    do_pref(SB)
            if bi == 1 and j0 + SB in sb_starts and (j0 + SB) not in prefs:
                do_pref(j0 + SB)
            ia_t, ib_t, mk_t, xr_t = prefs[j0]
            g = ga + gb
            slab = slabp.tile([P, g, P], dt.bfloat16, tag="slab", name="slab")
            if ga:
                nc.gpsimd.dma_gather(
                    slab[:, 0:ga, :], tabA[:, :], ia_t[:, 8 * la:8 * (la + ga)],
                    P * ga, nreg(P * ga), P, single_packet=False)
            if gb:
                nc.gpsimd.dma_gather(
                    slab[:, ga:g, :], tabB[:, :], ib_t[:, 8 * lb:8 * (lb + gb)],
                    P * gb, nreg(P * gb), P, single_packet=False)

            gp = g // PAIR
            ttT = tpose.tile([P, gp, P], dt.bfloat16, tag="ttT", name="ttT")
            uT = tpose.tile([P, gp, P], dt.bfloat16, tag="uT", name="uT")
            tt = None
            if not INPLACE:
                tt = tpose.tile([P, g, FU], dt.bfloat16, tag="tt", name="tt")
            exv = exvp.tile([P, g, H], dt.bfloat16, tag="exv", name="exv")
            chunks = [(s0, min(CHKS, g - s0)) for s0 in range(0, g, CHKS)]
            for (s0, sn) in chunks:
                sl_c = slab[:, s0:s0 + sn, 0:FU]
                xr_b = xr_t[:, bi, :].unsqueeze(1).to_broadcast([P, sn, FU])
                if INPLACE:
                    nc.vector.tensor_tensor(sl_c, sl_c, xr_b, op.add)
                    tt_c = sl_c
                else:
                    tt_c = tt[:, s0:s0 + sn, :]
                    nc.vector.tensor_tensor(tt_c, sl_c, xr_b, op.add)

                # ttT[pf, y, d] = tt[d, y*PAIR + pf//FU, pf%FU]
                c0, cn = s0 // PAIR, sn // PAIR
                ttT_c = ttT[:, c0:c0 + cn, :]
                uT_c = uT[:, c0:c0 + cn, :]
                nc.sync.dma_start_transpose(
                    ttT_c, tt_c.rearrange("p g f -> p (g f)"))
                nc.scalar.activation(uT_c, ttT_c, AF.Abs)

                ps_s = spsum.tile([P, cn * AW], dt.float32, tag="ps_s",
                                  name="ps_s")
                for q in range(cn):
                    o = q * AW
                    nc.tensor.matmul(
                        ps_s[:, o:o + AW], ttT[:, c0 + q, :],
                        att_s[:, 0:AW], start=True, stop=False)
                    nc.tensor.matmul(
                        ps_s[:, o:o + AW], uT[:, c0 + q, :],
                        att_s[:, AW:2 * AW], start=False, stop=True)
                nc.scalar.activation(
                    exv[:, s0:s0 + sn, :].rearrange("p g h -> p (g h)"),
                    ps_s[:], AF.Exp)
            state[jj] = dict(slab=slab, exv=exv, g=g, chunks=chunks,
                             mk_t=mk_t, xr_t=xr_t, bi=bi, lg=lg)

        def p2(blk):
            """per chunk: pad-mask exv, alpha-weight slab (in place)."""
            jj = blk[0]
            st = state[jj]
            slab, exv = st["slab"], st["exv"]
            mk_t, lg = st["mk_t"], st["lg"]
            for (s0, sn) in st["chunks"]:
                exv_c = exv[:, s0:s0 + sn, :]
                nc.vector.tensor_tensor(
                    exv_c, exv_c, mk_t[:, lg + s0:lg + s0 + sn, :], op.mult)
                sl_c = slab[:, s0:s0 + sn, 0:FU]
                nc.vector.tensor_tensor(
                    sl_c.rearrange("p g (c h) -> p g c h", h=H),
                    sl_c.rearrange("p g (c h) -> p g c h", h=H),
                    exv_c.unsqueeze(2).to_broadcast([P, sn, C, H]),
                    op.mult)

        def p3(blk):
            """agg/denom matmuls, normalization + bias (+ELU / softmax)."""
            jj = blk[0]
            st = state.pop(jj)
            slab, exv, g = st["slab"], st["exv"], st["g"]
            xr_t, bi = st["xr_t"], st["bi"]
            ps = psum.tile([P, FU], dt.float32, tag="ps", name="ps")
            dnp = psum.tile([P, H], dt.float32, tag="dnp", name="dnp")
            for q in range(g):
                nc.tensor.matmul(ps[:], id_s[:], slab[:, q, 0:FU],
                                 start=(q == 0), stop=(q == g - 1))
                nc.tensor.matmul(dnp[:], id_s[:], exv[:, q, :],
                                 start=(q == 0), stop=(q == g - 1))

            dn = small.tile([P, H], dt.float32, tag="dn", name="dn")
            nc.vector.tensor_scalar_add(dn[:], dnp[:], EPS)
            rd = small.tile([P, H], dt.float32, tag="rd", name="rd")
            nc.vector.reciprocal(rd[:], dn[:])
            ov = small.tile([P, FU], dt.float32, tag="ov", name="ov")
            nc.vector.tensor_tensor(
                ov[:].rearrange("p (c h) -> p c h", h=H),
                ps[:].rearrange("p (c h) -> p c h", h=H),
                rd[:].unsqueeze(1).to_broadcast([P, C, H]),
                op.mult)
            if INPLACE:
                # agg included xr*denom: subtract xr*(denom/(denom+eps))
                t2c = small.tile([P, H], dt.float32, tag="t2c", name="t2c")
                nc.vector.tensor_tensor(t2c[:], dnp[:], rd[:], op.mult)
                oc = small.tile([P, FU], dt.float32, tag="oc", name="oc")
                nc.vector.tensor_tensor(
                    oc[:].rearrange("p (c h) -> p c h", h=H),
                    xr_t[:, bi, :].rearrange("p (c h) -> p c h", h=H),
                    t2c[:].unsqueeze(1).to_broadcast([P, C, H]),
                    op.mult)
                nc.vector.tensor_tensor(ov[:], ov[:], oc[:], op.subtract)
            ob = small.tile([P, FU], dt.float32, tag="ob", name="ob")
            nc.vector.tensor_tensor(ob[:], ov[:], bias_s[:], op.add)

            if layer == 1:
                # ELU(x) = max(x, exp(min(x, 0)) - 1)
                mm_t = small.tile([P, FU], dt.float32, tag="mmt", name="mm_t")
                nc.vector.tensor_scalar_min(mm_t[:], ob[:], 0.0)
                em = small.tile([P, FU], dt.float32, tag="em", name="em")
                nc.scalar.activation(em[:], mm_t[:], AF.Exp)
                hg = small.tile([P, FU], dt.float32, tag="hg", name="hg")
                nc.vector.scalar_tensor_tensor(
                    hg[:], em[:], -1.0, ob[:], op.add, op.max)
                nc.sync.dma_start(out_d[jj * P:(jj + 1) * P, :], hg[:])
            else:
                yb = persist.tile([P, FU], dt.float32, tag=f"y{jj}",
                                  name=f"y{jj}")
                nc.vector.tensor_copy(yb[:], ob[:])
                yr = yb[:, 0:H * DOUT]      # (c,h): cols 0:56 are real
                mx2 = mx_all[:, jj:jj + 1]
                nc.vector.tensor_reduce(mx2, yr, mybir.AxisListType.X, op.max)
                mxn = small.tile([P, 1], dt.float32, tag="mxn", name="mxn")
                nc.vector.tensor_scalar_mul(mxn[:], mx2, -1.0)
                et = small.tile([P, H * DOUT], dt.float32, tag="et", name="et")
                nc.scalar.activation(et[:], yr, AF.Exp, bias=mxn[:])
                nc.vector.tensor_reduce(s_all[:, jj:jj + 1], et[:],
                                        mybir.AxisListType.X, op.add)
                y_tiles[jj] = yb

        for j in range(NB):
            p1(binfo[j])
            p2(binfo[j])
            p3(binfo[j])

        if layer == 2:
            # ln(S) via exponent/mantissa split (no Ln in the exp act table):
            # ln(S) = (e - 127)*ln2 + poly(m), m in [1, 2)
            C5, C4, C3, C2_, C1, C0 = (0.030102625011658456,
                                       -0.2806325404494927,
                                       1.1048082361987304,
                                       -2.4208125632180866,
                                       3.4982279012091095,
                                       -1.9316715417207186)
            bits = s_all[:].bitcast(dt.int32)
            ei = persist.tile([P, NB], dt.int32)
            nc.vector.tensor_scalar(ei[:], bits, 23, None,
                                    op.arith_shift_right)
            ef = persist.tile([P, NB], dt.float32)
            nc.vector.tensor_copy(ef[:], ei[:])
            mi = persist.tile([P, NB], dt.int32)
            nc.vector.tensor_scalar(mi[:], bits, 0x007FFFFF, 0x3F800000,
                                    op.bitwise_and, op.bitwise_or)
            mf = mi[:].bitcast(dt.float32)
            pp = persist.tile([P, NB], dt.float32)
            nc.vector.tensor_scalar(pp[:], mf, C5, C4, op.mult, op.add)
            qq = persist.tile([P, NB], dt.float32)
            for ck in (C3, C2_, C1, C0):
                nc.vector.tensor_tensor(qq[:], pp[:], mf, op.mult)
                nc.vector.tensor_scalar_add(pp[:], qq[:], ck)
            # ct = mx + (e-127)*ln2 + poly(m)
            lnm = pp
            ct_all = persist.tile([P, NB], dt.float32)
            nc.vector.scalar_tensor_tensor(
                ct_all[:], ef[:], 0.6931471805599453, lnm[:],
                op.mult, op.add)
            ct2 = persist.tile([P, NB], dt.float32)
            nc.vector.scalar_tensor_tensor(
                ct2[:], ct_all[:], -127.0 * 0.6931471805599453, mx_all[:],
                op.add, op.add)
            for j in range(NB):
                of = small.tile([P, FOUT], dt.float32, tag="of", name="of")
                nc.vector.tensor_scalar_sub(of[:], y_tiles[j][:],
                                            ct2[:, j:j + 1])
                nc.sync.dma_start(out_d[j * P:(j + 1) * P, :], of[:])
    nc.compile()
    return nc


# ---------------------------------------------------------------------------
# runner
# ---------------------------------------------------------------------------

_state = {}


def _run(nc, in_maps, trace=False):
    from concourse.bass_utils import run_bass_kernel_spmd
    return run_bass_kernel_spmd(nc, in_maps, core_ids=list(range(NCORES)),
                                trace=trace)


def _bcast_rows(v, rows=P):
    """[n] -> [rows, n] replicated, contiguous."""
    return np.ascontiguousarray(np.broadcast_to(np.asarray(v)[None, :],
                                                (rows, len(v))))


def _perm_ch(c, h):
    """col permutation: new[(ci*h) + hi] = old[hi*c + ci]"""
    idx = np.arange(c * h).reshape(c, h)      # new (c, h)
    old = idx % h * c + idx // h              # old index h*c + c
    return old.reshape(-1)


def _attc_tables(att, C, PAIR):
    """[128, 2*AW] bf16: [0.6 attd (| paired) | 0.4 attd (| paired)]."""
    AW = 8 * PAIR
    FUp = C * H
    a6 = np.zeros((P, AW), _f32)
    a4 = np.zeros((P, AW), _f32)
    att = np.asarray(att, _f32)
    for par in range(PAIR):
        for c in range(C):
            for h in range(H):
                f = par * FUp + c * H + h
                a6[f, par * 8 + h] = 0.6 * att[h, c]
                a4[f, par * 8 + h] = 0.4 * att[h, c]
    return np.concatenate([a6, a4], axis=1).astype(_bf16)


def kernel(x, edge_index, Wl1, bl1, Wr1, br1, att1, bias1,
           Wl2, bl2, Wr2, br2, att2, bias2, _trace=False, _times=None,
           _inmaps=None):
    x = np.asarray(x, _f32)
    edge_index = np.asarray(edge_index)

    g = _prep_graph(edge_index)
    members, GA, GB = g["members"], g["GA"], g["GB"]

    ckey = (tuple(GA), tuple(GB))
    if _state.get("ckey") != ckey:
        _state["ckey"] = ckey
        _state["nc_t1"] = _build_transform(2 * F1, F1, F1, F1)
        _state["nc_t2"] = _build_transform(2 * F2P, P, F2P, F2P)
        _state["nc_e1"] = _build_edge(1, GA, GB, g["sumGA"], g["sumGB"], g["sumG"])
        _state["nc_e2"] = _build_edge(2, GA, GB, g["sumGA"], g["sumGB"], g["sumG"])

    id128 = np.eye(P, dtype=_bf16)
    p1 = _perm_ch(DH, H)          # layer-1 (c,h) permutation of 128 cols
    p2 = _perm_ch(C2, H)          # layer-2 (c,h) permutation of 64 cols

    def gather_nodes(arr, mem):
        flat = mem.reshape(-1)
        out = arr[np.maximum(flat, 0)]
        out[flat < 0] = 0
        return out

    def trace_run(key, nc, in_maps):
        if _inmaps is not None:
            _inmaps[key] = in_maps
        r = _run(nc, in_maps, trace=_trace)
        if _times is not None:
            _times[key] = r.exec_time_ns
        return r.results

    # ---- T1 ----  (xl/xr columns in (c,h) order)
    Wl1p = np.asarray(Wl1, _f32)[:, p1]
    Wr1p = np.asarray(Wr1, _f32)[:, p1]
    bl1p = np.asarray(bl1, _f32)[p1]
    br1p = np.asarray(br1, _f32)[p1]
    W1 = np.concatenate([Wl1p, Wr1p], axis=1)                  # [128, 256]
    B1t = _bcast_rows(np.concatenate([bl1p, br1p]))
    t1_maps = []
    for k in range(NCORES):
        xg = gather_nodes(x, members[k])                       # [6272, 128]
        t1_maps.append({"xT": np.ascontiguousarray(xg.T),
                        "Wcat": W1, "Bcat": B1t})
    r1 = trace_run("t1", _state["nc_t1"], t1_maps)

    # assemble layer-1 gather table
    tab1 = np.zeros((NPAD, P), _bf16)
    for k in range(NCORES):
        flat = members[k].reshape(-1)
        ok = flat >= 0
        tab1[flat[ok]] = r1[k]["xl"][ok]
    tab1A = np.ascontiguousarray(tab1[:SPLIT])
    tab1B = np.ascontiguousarray(tab1[SPLIT:])

    # ---- E1 ----
    attc1 = _attc_tables(att1, DH, 1)
    bias1_t = _bcast_rows(np.asarray(bias1, _f32)[p1])
    e1_maps = []
    for k in range(NCORES):
        e1_maps.append({"tabA": tab1A, "tabB": tab1B,
                        "xr": r1[k]["xr"],
                        "idxA": g["idxA"][k], "idxB": g["idxB"][k],
                        "mk01": g["mk01"][k],
                        "attc": attc1, "biasT": bias1_t, "idT": id128})
    re1 = trace_run("e1", _state["nc_e1"], e1_maps)

    # ---- T2 ----  (input h is in (c,h) order -> permute W rows by p1;
    #                output xl2/xr2 cols in layer-2 (c,h) order)
    Wl2f = np.zeros((P, F2P), _f32)
    Wl2f.reshape(P, H, C2)[:, :, :DOUT] = np.asarray(Wl2, _f32).reshape(P, H, DOUT)
    Wr2f = np.zeros((P, F2P), _f32)
    Wr2f.reshape(P, H, C2)[:, :, :DOUT] = np.asarray(Wr2, _f32).reshape(P, H, DOUT)
    Wl2p = Wl2f[p1][:, p2]
    Wr2p = Wr2f[p1][:, p2]
    W2 = np.ascontiguousarray(np.concatenate([Wl2p, Wr2p], axis=1))  # [128,128]
    bl2f = np.zeros(F2P, _f32)
    bl2f.reshape(H, C2)[:, :DOUT] = np.asarray(bl2, _f32).reshape(H, DOUT)
    br2f = np.zeros(F2P, _f32)
    br2f.reshape(H, C2)[:, :DOUT] = np.asarray(br2, _f32).reshape(H, DOUT)
    B2t = _bcast_rows(np.concatenate([bl2f[p2], br2f[p2]]))
    t2_maps = []
    for k in range(NCORES):
        t2_maps.append({"xT": np.ascontiguousarray(re1[k]["out"].T),
                        "Wcat": W2, "Bcat": B2t})
    r2 = trace_run("t2", _state["nc_t2"], t2_maps)

    tab2 = np.zeros((NPAD, P), _bf16)
    for k in range(NCORES):
        flat = members[k].reshape(-1)
        ok = flat >= 0
        tab2[flat[ok]] = r2[k]["xl"][ok]
    tab2A = np.ascontiguousarray(tab2[:SPLIT])
    tab2B = np.ascontiguousarray(tab2[SPLIT:])

    # ---- E2 ----
    att2f = np.zeros((H, C2), _f32)
    att2f[:, :DOUT] = np.asarray(att2, _f32)
    attc2 = _attc_tables(att2f, C2, 2)
    bias2f = np.zeros(F2P, _f32)
    bias2f.reshape(H, C2)[:, :DOUT] = np.asarray(bias2, _f32).reshape(H, DOUT)
    bias2_t = _bcast_rows(bias2f[p2])
    e2_maps = []
    for k in range(NCORES):
        e2_maps.append({"tabA": tab2A, "tabB": tab2B,
                        "xr": r2[k]["xr"],
                        "idxA": g["idxA"][k], "idxB": g["idxB"][k],
                        "mk01": g["mk01"][k],
                        "attc": attc2, "biasT": bias2_t, "idT": id128})
    re2 = trace_run("e2", _state["nc_e2"], e2_maps)

    # un-permute layer-2 (c,h) cols back to (h, c-real) on host
    inv2 = np.empty(F2P, np.int64)
    inv2[p2] = np.arange(F2P)       # old index -> position in (c,h) layout
    out = np.zeros((N, H * DOUT), _f32)
    sel = inv2.reshape(H, C2)[:, :DOUT].reshape(-1)
    for k in range(NCORES):
        flat = members[k].reshape(-1)
        ok = flat >= 0
        out[flat[ok]] = re2[k]["out"][ok][:, sel]
    return out
